# revision 4
# baseline (speedup 1.0000x reference)
"""AxialLinearAttention Trainium2 kernel, v2.

Data-parallel over batch across 8 NeuronCores. Per core (BC=32, T=4096
tokens), feature-transposed activation layout (X^T: features on
partitions, tokens free) so all 8 projections are dense bf16
(128x128)@(128x512) matmul chains at full PE rate.

v2 over baseline:
 - 6 QKV weight matrices live resident in SBUF as bf16 (loaded once in
   a prologue); only the 2 out-projection weights stream per super-tile
   from a bf16 DRAM scratch image -> no weight DMA in the steady state.
 - all activations bf16 (xt/qt/kt/v/at/mid/fin), halving SBUF and
   giving DVE 2x modes where possible.
 - Q/K/V emission interleaved per j-chunk; scores use a single wide
   [128,1024] PSUM tile per head-pair with one fused masked eviction.
 - evictions balanced across Scalar and DVE to keep the PE stream
   gapless (PE p-state!).
 - DMAs issued from the otherwise-idle GPSIMD queue (25ns seq cost vs
   565ns on SP).
 - elu1(x) = min(exp(x), 1) + relu(x)  (exact; two forms used to
   balance engines: Q scalar-heavy, K vector-heavy).
"""

import os
import sys

sys.path.insert(0, "/opt/trn_rl_repo")

import numpy as np

import concourse.bacc as bacc
import concourse.bass as bass
import concourse.mybir as mybir
import concourse.tile as tile
from concourse.masks import make_identity

F32 = mybir.dt.float32
F32R = mybir.dt.float32r
BF16 = mybir.dt.bfloat16
AF = mybir.ActivationFunctionType
ALU = mybir.AluOpType

B, FG, ANT, D = 256, 4, 32, 1024
H, DK = 16, 64
NCORES = 8
P = 128
NPT = D // P  # 8 feature partition-tiles

W_NAMES = [
    "ant_q_w", "ant_k_w", "ant_v_w", "ant_out_w",
    "freq_q_w", "freq_k_w", "freq_v_w", "freq_out_w",
]
QKV_NAMES = ["ant_q_w", "ant_k_w", "ant_v_w",
             "freq_q_w", "freq_k_w", "freq_v_w"]
OUT_NAMES = ["ant_out_w", "freq_out_w"]


def asf32(ap):
    return ap.bitcast(F32)


def _emit_kernel(nc, tc, ctx, BC):
    T = BC * FG * ANT          # 4096 tokens per core
    ST = 512                   # tokens per super-tile
    NST = T // ST              # 8
    SL = ST // P               # 4 chunks (=batch rows) per super-tile
    W2 = 2 * ST                # wide scores width (two head-parities)

    x_d = nc.dram_tensor("x", [T, D], F32R, kind="ExternalInput").ap()
    w_d = {n: nc.dram_tensor(n, [D, D], F32R, kind="ExternalInput").ap()
           for n in W_NAMES}
    out_d = nc.dram_tensor("out", [T, D], F32, kind="ExternalOutput").ap()

    # ---- pools ----
    const_pool = ctx.enter_context(tc.tile_pool(name="const", bufs=1))
    wres_p = ctx.enter_context(tc.tile_pool(name="wres", bufs=1))
    wo_p = ctx.enter_context(tc.tile_pool(name="wop", bufs=2))
    xs_p = ctx.enter_context(tc.tile_pool(name="xs", bufs=1))
    big = ctx.enter_context(tc.tile_pool(name="big", bufs=1))
    sm_p = ctx.enter_context(tc.tile_pool(name="smp", bufs=2))
    eb_p = ctx.enter_context(tc.tile_pool(name="ebp", bufs=2))
    ostage_p = ctx.enter_context(tc.tile_pool(name="ostage", bufs=2))
    dram_p = ctx.enter_context(tc.tile_pool(name="drams", bufs=1, space="DRAM"))
    ps_pj = ctx.enter_context(tc.tile_pool(name="ps_pj", bufs=4, space="PSUM"))
    ps_sc = ctx.enter_context(tc.tile_pool(name="ps_sc", bufs=2, space="PSUM"))

    # ---- constants ----
    ident = const_pool.tile([P, P], F32)
    make_identity(nc, ident)
    identr = const_pool.tile([P, P], F32R)
    nc.scalar.activation(identr, ident, AF.Copy)
    identb = const_pool.tile([P, P], BF16)
    nc.vector.tensor_copy(identb, ident)

    # wide bf16 masks covering both parities (cols: par*ST + c*P)
    # ant: tokens grouped in contiguous 32-blocks; freq: l' ~ l mod 32
    mask_w = {}
    for nmm in ("ant", "freq"):
        mw = const_pool.tile([P, W2], BF16, name=f"mask_{nmm}", tag=f"mask_{nmm}")
        nc.gpsimd.memset(mw, 0.0)
        for rep in range(W2 // P):
            for g in range(4):
                if nmm == "ant":
                    nc.gpsimd.memset(
                        mw[32 * g:32 * g + 32,
                           rep * P + 32 * g:rep * P + 32 * g + 32], 1.0)
                else:
                    for b2 in range(4):
                        nc.vector.tensor_copy(
                            mw[32 * g:32 * g + 32,
                               rep * P + 32 * b2:rep * P + 32 * b2 + 32],
                            ident[0:32, 0:32])
        mask_w[nmm] = mw

    # ---- resident QKV weights, bf16, layout col = i*1024 + j*128 ----
    wres = {}
    for n in QKV_NAMES:
        wt = wres_p.tile([P, NPT * D], BF16, tag=f"w_{n}", name=f"w_{n}")
        for i in range(NPT):
            xs = xs_p.tile([P, D], F32R, tag=f"xs{i % SL}", name=f"xs{i % SL}")
            nc.gpsimd.dma_start(xs, w_d[n][i * P:(i + 1) * P, :])
            nc.scalar.activation(
                wt[:, i * D:(i + 1) * D], asf32(xs), AF.Copy)
        wres[n] = wt

    # ---- persistent activation tiles ----
    xt = [big.tile([P, ST], BF16, tag=f"xt{i}", name=f"xt{i}")
          for i in range(NPT)]
    qt = [big.tile([P, ST], BF16, tag=f"qt{i}", name=f"qt{i}")
          for i in range(NPT)]
    kt = [big.tile([P, ST], BF16, tag=f"kt{i}", name=f"kt{i}")
          for i in range(NPT)]
    vv = [big.tile([P, D], BF16, tag=f"v{sl}", name=f"v{sl}")
          for sl in range(SL)]
    at = [big.tile([P, ST], BF16, tag=f"at{i}", name=f"at{i}")
          for i in range(NPT)]
    mid = [big.tile([P, ST], BF16, tag=f"mid{i}", name=f"mid{i}")
           for i in range(NPT)]
    fin = [big.tile([P, ST], BF16, tag=f"fin{i}", name=f"fin{i}")
           for i in range(NPT)]

    # ---- out-proj weights -> bf16 DRAM scratch, layout col = jc*1024+i*128
    # (vv tiles double as the bf16 cast staging; they are idle here)
    wo_dram = {}
    for n in OUT_NAMES:
        scratch = dram_p.tile([P, NPT * D], BF16, tag=f"wo_{n}", name=f"wo_{n}")
        for i in range(NPT):
            xs = xs_p.tile([P, D], F32R, tag=f"xs{i % SL}", name=f"xs{i % SL}")
            nc.gpsimd.dma_start(xs, w_d[n][i * P:(i + 1) * P, :])
            wc = vv[i % SL]
            nc.scalar.activation(wc, asf32(xs), AF.Copy)
            nc.gpsimd.dma_start(
                scratch.rearrange("p (jc w) -> p jc w", w=D)[
                    :, :, i * P:(i + 1) * P],
                wc.rearrange("p (jc q) -> p jc q", q=P))
        wo_dram[n] = scratch

    def x_load(s):
        """Issue the 4 x-chunk DMAs for super-tile s."""
        tiles = []
        for sl in range(SL):
            xs = xs_p.tile([P, D], F32R, tag=f"xs{sl}", name=f"xs{sl}")
            nc.gpsimd.dma_start(
                xs, x_d[s * ST + sl * P:s * ST + (sl + 1) * P, :])
            tiles.append(xs)
        return tiles

    def t_in(xs_tiles):
        """x chunks -> X^T bf16 feature tiles (PE transpose, f32r)."""
        for p2 in range(NPT // 2):
            tp = ps_sc.tile([P, W2], F32, tag="sc", name="tp")
            tpr = tp.bitcast(F32R)
            for k in range(2):
                i = 2 * p2 + k
                for sl in range(SL):
                    nc.tensor.transpose(
                        tpr[:, k * ST + sl * P:k * ST + (sl + 1) * P],
                        xs_tiles[sl][:, i * P:(i + 1) * P], identr)
            for k in range(2):
                i = 2 * p2 + k
                src = asf32(tpr[:, k * ST:(k + 1) * ST])
                if i % 2 == 0:
                    nc.vector.tensor_copy(xt[i], src)
                else:
                    nc.scalar.activation(xt[i], src, AF.Copy)

    def triple(blk, src, j):
        """One interleaved Q/K/V projection chunk from src."""
        wq, wk, wv = (wres[f"{blk}_q_w"], wres[f"{blk}_k_w"],
                      wres[f"{blk}_v_w"])
        if True:
            # --- Q[j] (scalar-heavy elu) ---
            ps = ps_pj.tile([P, ST], F32, tag="pj", name="psq")
            for i in range(NPT):
                nc.tensor.matmul(
                    ps, lhsT=wq[:, i * D + j * P:i * D + (j + 1) * P],
                    rhs=src[i], start=(i == 0), stop=(i == NPT - 1))
            e = eb_p.tile([P, ST], BF16, tag="e", name="e")
            nc.scalar.activation(e, ps, AF.Exp)
            r = eb_p.tile([P, ST], BF16, tag="rt", name="r")
            nc.scalar.activation(r, ps, AF.Relu)
            nc.vector.scalar_tensor_tensor(
                qt[j], e, 1.0, r, op0=ALU.min, op1=ALU.add)
            # --- K[j] (vector-heavy elu) ---
            ps = ps_pj.tile([P, ST], F32, tag="pj", name="psk")
            for i in range(NPT):
                nc.tensor.matmul(
                    ps, lhsT=wk[:, i * D + j * P:i * D + (j + 1) * P],
                    rhs=src[i], start=(i == 0), stop=(i == NPT - 1))
            e = eb_p.tile([P, ST], BF16, tag="e", name="e")
            nc.scalar.activation(e, ps, AF.Exp)
            t = eb_p.tile([P, ST], BF16, tag="rt", name="t")
            nc.vector.tensor_scalar(t, ps, 1.0, 1.0, op0=ALU.add, op1=ALU.max)
            nc.vector.tensor_tensor(kt[j], t, e, op=ALU.min)
            # --- V unit j -> natural (token, feature) layout ---
            j2, sl = j // SL, j % SL
            ps = ps_pj.tile([P, ST], F32, tag="pj", name="psv")
            for i in range(NPT):
                nc.tensor.matmul(
                    ps, lhsT=src[i][:, sl * P:(sl + 1) * P],
                    rhs=wv[:, i * D + j2 * ST:i * D + (j2 + 1) * ST],
                    start=(i == 0), stop=(i == NPT - 1))
            nc.scalar.activation(vv[sl][:, j2 * ST:(j2 + 1) * ST], ps, AF.Copy)

    def att_sc(hp, mask):
        """Scores + masked eviction for one head-pair -> sm tile."""
        sp = ps_sc.tile([P, W2], F32, tag="sc", name="sp")
        for par in range(2):
            off = 64 * par
            for c in range(SL):
                nc.tensor.matmul(
                    sp[:, par * ST + c * P:par * ST + (c + 1) * P],
                    lhsT=kt[hp][off:off + 64, c * P:(c + 1) * P],
                    rhs=qt[hp][off:off + 64, c * P:(c + 1) * P],
                    start=True, stop=True)
        sm = sm_p.tile([P, W2], BF16, tag="sm", name="sm", bufs=3)
        nc.vector.tensor_tensor(sm, sp, mask, op=ALU.mult)
        return sm

    def att_av(hp, sm):
        """A^T[hp] = V^T @ sm."""
        ps = ps_pj.tile([P, ST], F32, tag="pj", name="psa")
        for par in range(2):
            off = 64 * par
            for c in range(SL):
                nc.tensor.matmul(
                    ps[off:off + 64, c * P:(c + 1) * P],
                    lhsT=vv[c][:, hp * P + off:hp * P + off + 64],
                    rhs=sm[:, par * ST + c * P:par * ST + (c + 1) * P],
                    start=True, stop=True)
        nc.scalar.activation(at[hp], ps, AF.Copy)

    # QKV triples interleaved with attention so projection matmuls hide
    # the DVE mask-eviction latency. sc(hp) needs qt/kt[hp] (triple hp);
    # av(hp<4) needs V units 0-3 (triple 3); av(hp>=4) needs triple 7.
    BLOCK_PLAN = [
        ("t", 0), ("t", 1), ("sc", 0), ("t", 2), ("sc", 1), ("t", 3),
        ("av", 0), ("sc", 2), ("t", 4), ("av", 1), ("sc", 3), ("t", 5),
        ("av", 2), ("sc", 4), ("t", 6), ("av", 3), ("sc", 5), ("sc", 6),
        ("t", 7), ("sc", 7), ("av", 4), ("av", 5), ("av", 6), ("av", 7),
    ]

    def block(blk, src, mask):
        sms = {}
        for kind, idx in BLOCK_PLAN:
            if kind == "t":
                triple(blk, src, idx)
            elif kind == "sc":
                sms[idx] = att_sc(idx, mask)
            else:
                att_av(idx, sms.pop(idx))

    def wo_load(blk):
        tiles = []
        for jh in range(2):
            wo = wo_p.tile([P, 4 * D], BF16, tag="wo", name="wo")
            nc.gpsimd.dma_start(
                wo, wo_dram[f"{blk}_out_w"][:, jh * 4 * D:(jh + 1) * 4 * D])
            tiles.append(wo)
        return tiles

    def outproj(wo_tiles, res, dst):
        """dst[j] = res[j] + Wo^T @ A^T[j]."""
        for j in range(NPT):
            wo = wo_tiles[j // 4]
            jc = j % 4
            ps = ps_pj.tile([P, ST], F32, tag="pj", name="pso")
            for i in range(NPT):
                nc.tensor.matmul(
                    ps, lhsT=wo[:, jc * D + i * P:jc * D + (i + 1) * P],
                    rhs=at[i], start=(i == 0), stop=(i == NPT - 1))
            nc.vector.tensor_tensor(dst[j], ps, res[j], op=ALU.add)

    def w_out(s):
        for sl in range(SL):
            tp = ps_pj.tile([P, ST], F32, tag="pj", name="tpo")
            tpb = tp.bitcast(BF16)
            for j in range(NPT):
                nc.tensor.transpose(
                    tpb[:, j * P:(j + 1) * P],
                    fin[j][:, sl * P:(sl + 1) * P], identb)
            ost = ostage_p.tile([P, D], F32, tag="os", name="ost")
            nc.scalar.activation(ost, tpb[:, 0:D], AF.Copy)
            nc.gpsimd.dma_start(
                out_d[s * ST + sl * P:s * ST + (sl + 1) * P, :], ost)

    # ================= main loop =================
    REP = int(os.environ.get("K_REPEAT", "1"))  # timing experiments only
    NIT = NST * REP
    xs_cur = x_load(0)
    prev_s = None
    for s_ in range(NIT):
        s = s_ % NST
        t_in(xs_cur)
        if prev_s is not None:
            w_out(prev_s)  # emitted after t_in(s): fills the boundary bubble
        wo_a = wo_load("ant")
        block("ant", xt, mask_w["ant"])
        outproj(wo_a, xt, mid)
        wo_f = wo_load("freq")
        if s_ + 1 < NIT:
            xs_cur = x_load((s_ + 1) % NST)
        block("freq", mid, mask_w["freq"])
        outproj(wo_f, mid, fin)
        prev_s = s
    w_out(prev_s)


def build(BC):
    from contextlib import ExitStack

    nc = bacc.Bacc("TRN2", target_bir_lowering=False, debug=False)
    with tile.TileContext(nc) as tc:
        with ExitStack() as ctx:
            _emit_kernel(nc, tc, ctx, BC)
    nc.compile()
    return nc


_CACHE = {}
last_results = None


def kernel(x, **inputs):
    """Full (unsharded) inputs -> full output. Shards batch across 8 cores."""
    global last_results
    from concourse.bass_utils import run_bass_kernel_spmd

    x = np.ascontiguousarray(np.asarray(x), dtype=np.float32)
    BC = B // NCORES
    if "nc" not in _CACHE:
        _CACHE["nc"] = build(BC)
    nc = _CACHE["nc"]

    weights = {n: np.ascontiguousarray(np.asarray(inputs[n]), dtype=np.float32)
               for n in W_NAMES}
    in_maps = []
    for k in range(NCORES):
        m = {"x": x[k * BC:(k + 1) * BC].reshape(BC * FG * ANT, D)}
        m.update(weights)
        in_maps.append(m)

    res = run_bass_kernel_spmd(nc, in_maps, core_ids=list(range(NCORES)))
    last_results = res
    out = np.empty((B, FG * ANT, D), dtype=np.float32)
    for k in range(NCORES):
        out[k * BC:(k + 1) * BC] = res.results[k]["out"].reshape(BC, FG * ANT, D)
    return out
